# revision 3
# baseline (speedup 1.0000x reference)
"""Trainium2 Bass kernel for a CRF layer (forward-score - gold-score NLL).

Strategy (per core, data-parallel over batch: 16 sequences/core x 8 cores):
  - emissions em' = Wext^T @ hx in bf16 (PSUM fp32), where hx carries two
    extra contraction rows folding  mask*b[i] - K[t,b]  into the matmul
    (K = host drift-normalizer; pad cols get -40 so exp(em') ~ 0).
  - x = exp(em') on ScalarE; row 64 of the x-plane is a host-shipped
    delta selector d[b,t0] = 1[len_b == t0].
  - Exp-space CRF recurrence, two-sided to halve the serial chain:
      fwd:  A_t = (E @ A_{t-1}) * x_t          (t = 1..L/2)
      bwd:  w_{t-1} = E^T (x_t * w_t) + Eend * d_t   (t = L..L/2+1)
    Pad steps annihilate columns; the END-projection is injected exactly:
    fwd captures Eend^T A_len via an extra weight column + delta row
    (accumulated into a PSUM bank by a K=1 matmul per step), bwd
    resurrects w = Eend at t = len via an extra contraction row.
  - score[b] = w_512^T A_512 + fwd-captured score; both summed in PSUM.
  - gold via one-hot matmuls: X = trans @ Oprev accumulated onto em' in
    PSUM, then sum(Om * (em' + X)) via DVE mul + strided reduce.
  - host: nll = log(score) - gold - trans[END, last_tag]  (the K-terms
    cancel exactly between the forward and gold scores).
"""

import numpy as np
import ml_dtypes
from contextlib import ExitStack

import concourse.bass as bass
import concourse.bacc as bacc
import concourse.tile as tile
import concourse.mybir as mybir
from concourse import bass_utils

bf16 = ml_dtypes.bfloat16
B, L_FULL, H, T = 128, 1024, 512, 64
BEGIN, END = 0, 1
NCORE, BC = 8, 16
CCOL = 512  # columns per emission chunk (= 32 timesteps x 16 seqs)


# --------------------------------------------------------------------------
# device program
# --------------------------------------------------------------------------

def build_nc(L=L_FULL):
    HALF = L // 2
    NCOL = L * BC
    NCH = NCOL // CCOL          # number of emission chunks
    FCH = NCH // 2              # chunks feeding the fwd chain
    f32 = mybir.dt.float32
    bfd = mybir.dt.bfloat16
    AX = mybir.AxisListType
    AL = mybir.AluOpType
    AF = mybir.ActivationFunctionType

    nc = bacc.Bacc("TRN2", target_bir_lowering=False, debug=False,
                   num_devices=NCORE)

    def dr(name, shape, dt, kind="ExternalInput"):
        return nc.dram_tensor(name, shape, dt, kind=kind).ap()

    hx = dr("hx", [H + 2, NCOL], bfd)
    wext = dr("wext", [H + 2, T], bfd)
    dpl = dr("dpl", [1, NCOL], f32)
    eft = dr("eft", [T, T + 1], f32)
    ebt = dr("ebt", [T + 1, T], f32)
    trt = dr("trt", [T, T], f32)
    om = dr("om", [T, NCOL], bfd)
    opv = dr("opv", [T, NCOL], f32)
    a0 = dr("a0", [T, BC], f32)
    w0e = dr("w0e", [T + 1, BC], f32)
    outp = dr("out", [2, BC], f32, kind="ExternalOutput")

    with tile.TileContext(nc) as tc, ExitStack() as ctx:
        const = ctx.enter_context(tc.tile_pool(name="const", bufs=1))
        empool = ctx.enter_context(tc.tile_pool(name="empool", bufs=2, space="PSUM"))
        chp = ctx.enter_context(tc.tile_pool(name="chp", bufs=1, space="PSUM"))
        hxp = ctx.enter_context(tc.tile_pool(name="hxp", bufs=3))
        omp = ctx.enter_context(tc.tile_pool(name="omp", bufs=2))
        opp = ctx.enter_context(tc.tile_pool(name="opp", bufs=2))
        up = ctx.enter_context(tc.tile_pool(name="up", bufs=2))
        redp = ctx.enter_context(tc.tile_pool(name="redp", bufs=2))
        apool = ctx.enter_context(tc.tile_pool(name="apool", bufs=3))
        vpool = ctx.enter_context(tc.tile_pool(name="vpool", bufs=3))

        # ---- constants ----
        wsb = []
        for k in range(4):
            wk = const.tile([128, T], bfd, name=f"wsb{k}")
            nc.sync.dma_start(out=wk, in_=wext[k * 128:(k + 1) * 128, :])
            wsb.append(wk)
        wk4 = const.tile([2, T], bfd, name="wsb4")
        nc.sync.dma_start(out=wk4, in_=wext[H:H + 2, :])
        wsb.append(wk4)

        eftt = const.tile([T, T + 1], f32, name="eftt")
        nc.sync.dma_start(out=eftt, in_=eft)
        ebtt = const.tile([T + 1, T], f32, name="ebtt")
        nc.sync.dma_start(out=ebtt, in_=ebt)
        trtt = const.tile([T, T], f32, name="trtt")
        nc.sync.dma_start(out=trtt, in_=trt)
        cst = const.tile([128, 1], f32, name="cst")
        nc.vector.memset(cst, 1.0)
        a0t = const.tile([T + 1, BC], f32, name="a0t")
        nc.sync.dma_start(out=a0t[0:T, :], in_=a0)
        w0t = const.tile([T + 1, BC], f32, name="w0t")
        nc.sync.dma_start(out=w0t, in_=w0e)
        gacc = const.tile([T, BC], f32, name="gacc")
        nc.vector.memset(gacc, 0.0)

        xbuf = const.tile([T + 1, NCOL], f32, name="xbuf")
        nc.sync.dma_start(out=xbuf[T:T + 1, :], in_=dpl)

        # chain PSUM tiles (persistent, manually rotated)
        fwdp = [chp.tile([T + 1, BC], f32, name=f"fwdp{i}") for i in range(2)]
        bwdp = [chp.tile([T + 1, BC], f32, name=f"bwdp{i}") for i in range(2)]
        scorep = chp.tile([1, BC], f32, name="scorep")
        goldp = chp.tile([1, BC], f32, name="goldp")
        for i in range(2):
            nc.vector.memset(bwdp[i][T:T + 1, :], 1.0)

        # ---- emission / gold chunk pipeline ----
        def emit_chunk(c):
            c0 = c * CCOL
            emp = empool.tile([T, CCOL], f32, name="emp", tag="emp")
            for k in range(5):
                r0 = k * 128
                r1 = min(r0 + 128, H + 2)
                n = r1 - r0
                tg = "hx" if k < 4 else "hx4"
                hxt = hxp.tile([n, CCOL], bfd, name=f"hxt{k}", tag=tg)
                nc.sync.dma_start(out=hxt, in_=hx[r0:r1, c0:c0 + CCOL])
                nc.tensor.matmul(emp, wsb[k][0:n, :], hxt,
                                 start=(k == 0), stop=(k == 4))
            # x = exp(em')  (must read emp before X accumulates onto it)
            nc.scalar.activation(out=xbuf[0:T, c0:c0 + CCOL], in_=emp, func=AF.Exp)
            # X = trans @ Oprev accumulated onto em' in PSUM
            opt = opp.tile([T, CCOL], f32, name="opt", tag="op")
            nc.sync.dma_start(out=opt, in_=opv[:, c0:c0 + CCOL])
            nc.tensor.matmul(emp, trtt, opt, start=False, stop=True,
                             skip_group_check=True)
            # gold partial: sum_t Om * (em' + X)
            omt = omp.tile([T, CCOL], bfd, name="omt", tag="om")
            nc.sync.dma_start(out=omt, in_=om[:, c0:c0 + CCOL])
            ut = up.tile([T, CCOL], f32, name="ut", tag="u")
            nc.vector.tensor_mul(ut, emp, omt)
            red = redp.tile([T, BC], f32, name="red", tag="red")
            nc.vector.tensor_reduce(
                out=red, in_=ut.rearrange("p (t b) -> p b t", b=BC),
                axis=AX.X, op=AL.add)
            nc.vector.tensor_add(gacc, gacc, red)

        # chunk pair order: fwd chunks ascending, bwd chunks descending
        pairs = [(g, NCH - 1 - g) for g in range(FCH)]
        emitted = 0

        def emit_pair():
            nonlocal emitted
            if emitted < len(pairs):
                emit_chunk(pairs[emitted][0])
                emit_chunk(pairs[emitted][1])
                emitted += 1

        emit_pair()
        emit_pair()

        # ---- the two serial chains ----
        Aprev = a0t
        rounds_per_chunk = CCOL // BC  # 32
        for t in range(1, HALF + 1):
            tp = L + 1 - t
            # fwd: P = [E^T | Eend]^T @ A_{t-1}
            Pf = fwdp[t % 2]
            nc.tensor.matmul(Pf, eftt, Aprev[0:T, :], start=True, stop=True)
            An = apool.tile([T + 1, BC], f32, name="A", tag="A")
            nc.vector.tensor_mul(An, Pf, xbuf[0:T + 1, (t - 1) * BC:t * BC])
            # accumulate delta-selected Eend^T A_len into scorep
            nc.tensor.matmul(scorep, cst[64:65, 0:1], An[T:T + 1, :],
                             start=(t == 1), stop=False,
                             tile_position=(64, 0), skip_group_check=True)
            # bwd: w_{tp-1} = E^T (x_tp * w_tp) + Eend * d_tp
            vn = vpool.tile([T + 1, BC], f32, name="v", tag="v")
            src = w0t if t == 1 else bwdp[(t - 1) % 2]
            nc.vector.tensor_mul(vn, src, xbuf[0:T + 1, (tp - 1) * BC:tp * BC])
            Pb = bwdp[t % 2]
            nc.tensor.matmul(Pb[0:T, :], ebtt, vn, start=True, stop=True)
            Aprev = An
            if t % rounds_per_chunk == 0:
                emit_pair()

        while emitted < len(pairs):
            emit_pair()

        # ---- finalization ----
        wlast = bwdp[HALF % 2]
        prod = const.tile([T, BC], f32, name="prod")
        nc.vector.tensor_mul(prod, wlast[0:T, :], Aprev[0:T, :])
        nc.tensor.matmul(scorep, cst[0:T, 0:1], prod, start=False, stop=True,
                         skip_group_check=True)
        nc.tensor.matmul(goldp, cst[0:T, 0:1], gacc, start=True, stop=True)
        stage_s = const.tile([1, BC], f32, name="stage_s")
        stage_g = const.tile([1, BC], f32, name="stage_g")
        nc.vector.tensor_copy(stage_s, scorep)
        nc.vector.tensor_copy(stage_g, goldp)
        nc.sync.dma_start(out=outp[0:1, :], in_=stage_s)
        nc.sync.dma_start(out=outp[1:2, :], in_=stage_g)

    nc.compile()
    return nc


# --------------------------------------------------------------------------
# host-side preparation
# --------------------------------------------------------------------------

def host_prep_shared(W, b, transition):
    E32 = np.exp(transition.astype(np.float32)).astype(np.float32)
    Ef64 = E32.astype(np.float64)
    Eend = E32[END, :].astype(np.float32)

    vE = np.ones(T)
    for _ in range(300):
        vE = Ef64 @ vE
        vE /= np.linalg.norm(vE)
    lamE = float(np.log((vE @ (Ef64 @ vE)) / (vE @ vE)))
    piL = np.ones(T)
    for _ in range(300):
        piL = Ef64.T @ piL
        piL /= np.linalg.norm(piL)
    pi = np.maximum(vE, 0) * np.maximum(piL, 0)
    pi = pi / pi.sum()
    c0 = 0.5 * (1.0 - float((pi ** 2).sum()))
    Wpi = (W.astype(np.float64) @ pi).astype(np.float32)

    Wext = np.empty((H + 2, T), dtype=bf16)
    Wext[:H] = W.astype(bf16)
    Wext[H] = np.asarray(b, np.float32).astype(bf16)
    Wext[H + 1] = np.float32(-1.0)

    eft = np.concatenate([E32.T, E32[END:END + 1, :].T], axis=1).astype(np.float32)
    ebt = np.concatenate([E32, E32[END:END + 1, :]], axis=0).astype(np.float32)
    trt = np.ascontiguousarray(transition.astype(np.float32).T)
    a0 = np.repeat(np.exp(transition[:, BEGIN].astype(np.float32))[:, None],
                   BC, axis=1).astype(np.float32)
    return dict(E32=E32, Eend=Eend, lamE=lamE, c0=c0, Wpi=Wpi,
                Wext=Wext, eft=eft, ebt=ebt, trt=trt, a0=a0)


def prep_core_inputs(c, hiddens, lens, tags, prep, L=L_FULL):
    NCOL = L * BC
    sl = slice(c * BC, (c + 1) * BC)
    hid = np.asarray(hiddens[sl], np.float32)
    ln = np.asarray(lens[sl]).astype(np.int64)
    tg = np.asarray(tags[sl]).astype(np.int64)

    t0s = np.arange(L)
    mask = (t0s[None, :] < ln[:, None]).astype(np.float32)  # [BC, L]
    Mbar = hid @ prep["Wpi"]
    sigma2 = (hid * hid).sum(axis=2) / H
    kappa = prep["lamE"] + prep["c0"] * sigma2 + Mbar
    Kfull = np.where(mask > 0, kappa, 40.0).astype(np.float32)

    hx = np.empty((H + 2, NCOL), dtype=bf16)
    hx[:H] = hid.transpose(2, 1, 0).reshape(H, NCOL).astype(bf16)
    hx[H] = mask.T.reshape(NCOL).astype(bf16)
    hx[H + 1] = Kfull.astype(bf16).T.reshape(NCOL)

    mcol = mask.T.reshape(NCOL)
    omv = (mcol[None, :] * (tg.T.reshape(NCOL)[None, :] ==
                            np.arange(T)[:, None])).astype(bf16)
    prevtag = np.concatenate([np.full((BC, 1), BEGIN, np.int64), tg[:, :-1]],
                             axis=1)
    opvv = (mcol[None, :] * (prevtag.T.reshape(NCOL)[None, :] ==
                             np.arange(T)[:, None])).astype(np.float32)

    dpl = (t0s[None, :, None] == ln[None, None, :]).astype(np.float32)
    dpl = dpl.reshape(1, L, BC)[:, :, :]  # d[0, t0, b]
    dpl = np.ascontiguousarray(dpl.reshape(1, NCOL))

    w0e = np.empty((T + 1, BC), np.float32)
    w0e[0:T] = prep["Eend"][:, None] * (ln == L).astype(np.float32)[None, :]
    w0e[T] = 1.0

    return dict(hx=np.ascontiguousarray(hx),
                wext=prep["Wext"],
                dpl=dpl,
                eft=prep["eft"], ebt=prep["ebt"], trt=prep["trt"],
                om=np.ascontiguousarray(omv),
                opv=np.ascontiguousarray(opvv),
                a0=prep["a0"], w0e=w0e)


def finalize(c, out2, lens, tags, transition):
    """out2: device [2, BC] -> per-core nll[BC]."""
    sl = slice(c * BC, (c + 1) * BC)
    ln = np.asarray(lens[sl]).astype(np.int64)
    tg = np.asarray(tags[sl]).astype(np.int64)
    meet = out2[0].astype(np.float64)
    gold = out2[1].astype(np.float32)
    last_tag = tg[np.arange(BC), ln - 1]
    gend = np.asarray(transition, np.float32)[END, last_tag]
    return (np.log(meet).astype(np.float32) - gold - gend).astype(np.float32)


_NC_CACHE = {}


def _get_nc(L=L_FULL):
    if L not in _NC_CACHE:
        _NC_CACHE[L] = build_nc(L)
    return _NC_CACHE[L]


def kernel(hiddens, lens, tags, W, b, transition):
    hiddens = np.asarray(hiddens, np.float32)
    lens_np = np.asarray(lens)
    tags_np = np.asarray(tags)
    W = np.asarray(W, np.float32)
    b = np.asarray(b, np.float32)
    transition = np.asarray(transition, np.float32)

    prep = host_prep_shared(W, b, transition)
    in_maps = [prep_core_inputs(c, hiddens, lens_np, tags_np, prep)
               for c in range(NCORE)]
    nc = _get_nc()
    res = bass_utils.run_bass_kernel_spmd(nc, in_maps,
                                          core_ids=list(range(NCORE)))
    nll = np.empty(B, np.float32)
    for c in range(NCORE):
        nll[c * BC:(c + 1) * BC] = finalize(
            c, res.results[c]["out"], lens_np, tags_np, transition)
    return nll
